# revision 28
# baseline (speedup 1.0000x reference)
# Additive self-attention (tanh-scored) Trainium2 Bass kernel.
#
# reference:
#   scores[b,i,j] = sum_d tanh(x[b,i,d] + x[b,j,d])     (B=4, N=1024, D=64)
#   out = softmax(scores, axis=-1) @ x
#
# Sharding: 8 cores = 4 batches x 2 query-halves. Each core computes 512
# query rows of one batch against all 1024 keys. Host-side prep is layout
# only (transpose/pack); all math happens on device.
#
# Per-core dataflow:
#  - xin [128, 1920] f32, one DMA (so consumers carry a single DMA-queue
#    semaphore wait -- walrus codegen allows only ONE sync wait per
#    instruction):
#      [:, 0:1024]  xkT2: [xk^T; xk^T] stacked (partition p = (r, d))
#      [:, 1024:1280] xqB: column t = concat(xq[2t, :], xq[2t+1, :])
#      [:, 1280:1408] identity (for PE transposes)
#      [:, 1408:1920] xk packed [128, kc, 64] for the AV matmul
#  - ebig [128, 254] float32r, second DMA: sliding-window 0/1 selection
#    matrix; window at 126-2s gives E_s[p, m] = (m == 2s + p//64).
#  - For query pair t: one ACTIVATE computes
#        T2[p, k] = tanh(xkT2[p, k] + xqB[p, t])     (FD = 1024, f32r out)
#    i.e. tanh(xq[i,d] + xk[k,d]) for i = 2t, 2t+1 -- the pairwise add is
#    fused into ACT's per-partition bias.
#  - PE reduces over d with accumulating matmuls (lhsT = E_s window f32r,
#    rhs = T2 f32r at full rate): 64 pairs accumulate into PSUM giving an
#    S block [128 q, 1024 k] (2 banks of 512).
#  - Softmax: no max-shift needed (|S| <= 64 so exp can't overflow fp32);
#    Exp on ACT, row-sums + reciprocal on DVE.
#  - AV: W normalized by 1/Z in-place on ACT, PE transposes of W chunks
#    (fp32) -> ScalarE copies PSUM->SBUF -> accumulate W^T x in fp32 ->
#    DVE copies PSUM->SBUF -> one DMA out.
#
# Engine-dependency discipline (ONE sync wait per instruction): deps on the
# same engine's semaphore merge, so each instruction may have fresh deps on
# at most one other engine/queue. Two dummy PE transposes at the start
# absorb the two input-DMA semaphores into the PE's clock.

from contextlib import ExitStack

import numpy as np

import concourse.bass as bass
import concourse.mybir as mybir
import concourse.tile as tile
from concourse.bass_utils import run_bass_kernel_spmd

B, N, D = 4, 1024, 64
NCORES = 8
Q = N // 2          # query rows per core = 512
P = 2 * D           # SBUF partitions used = 128
QB = 128            # query rows per output block
PPB = QB // 2       # query pairs per block = 64
TP = Q // 2         # total query pairs per core = 256
NB = Q // QB        # output blocks per core = 4
KC = N // 128       # key chunks = 8
EBASE = 2 * (PPB - 1)   # 126
EW = EBASE + QB         # ebig width = 254

XKT2_OFF = 0
XQB_OFF = N                  # 1024
ID_OFF = XQB_OFF + TP        # 1280
XKAV_OFF = ID_OFF + 128      # 1408
SU_OFF = XKAV_OFF + KC * D   # 1920
XIN_W = SU_OFF + 128         # 2048

F32 = mybir.dt.float32
F32R = mybir.dt.float32r


def _build_bass():
    nc = bass.Bass(trn_type="TRN2")

    xin = nc.dram_tensor("xin", [P, XIN_W], F32, kind="ExternalInput")
    ebig = nc.dram_tensor("ebig", [P, EW], F32R, kind="ExternalInput")
    out = nc.dram_tensor("out", [Q, D], F32, kind="ExternalOutput")

    with tile.TileContext(nc) as tc, ExitStack() as ctx:
        singles = ctx.enter_context(tc.tile_pool(name="singles", bufs=1))
        t2pool = ctx.enter_context(tc.tile_pool(name="t2", bufs=4))
        spool = ctx.enter_context(tc.tile_pool(name="spsum", bufs=2, space="PSUM"))
        wtps = ctx.enter_context(tc.tile_pool(name="wtps", bufs=2, space="PSUM"))
        avps = ctx.enter_context(tc.tile_pool(name="avps", bufs=1, space="PSUM"))
        junkps = ctx.enter_context(tc.tile_pool(name="junkps", bufs=1, space="PSUM"))
        sm = ctx.enter_context(tc.tile_pool(name="sm", bufs=4))
        wpool = ctx.enter_context(tc.tile_pool(name="w", bufs=5))
        opool = ctx.enter_context(tc.tile_pool(name="o", bufs=1))

        xin_s = singles.tile([P, XIN_W], F32)
        nc.sync.dma_start(out=xin_s, in_=xin[:, :])
        ebig_s = singles.tile([P, EW], F32R)
        nc.sync.dma_start(out=ebig_s, in_=ebig[:, :])

        xkT2_v = xin_s[:, XKT2_OFF:XKT2_OFF + N]
        xqB_v = xin_s[:, XQB_OFF:XQB_OFF + TP]
        id_v = xin_s[:, ID_OFF:ID_OFF + 128]
        xkAV_v = xin_s[:, XKAV_OFF:XKAV_OFF + KC * D].rearrange(
            "p (c d) -> p c d", c=KC
        )
        su_v = xin_s[:, SU_OFF:SU_OFF + 128]
        obig = opool.tile([128, NB, D], F32)

        # Dummy PE transposes: absorb the two DMA semaphores into PE's clock
        # so later matmuls never need a DMA wait on top of their ACT wait.
        junk = junkps.tile([2, 1], F32)
        nc.tensor.transpose(
            junk, ebig_s[:, 0:2].bitcast(F32), ebig_s[:, 0:1].bitcast(F32)
        )
        nc.tensor.transpose(junk, id_v[:, 0:2], id_v[:, 0:1])
        # DVE absorber for the xin DMA semaphore (first DVE reader of xin)
        dve_touch = sm.tile([128, 1], F32, tag="touch")
        nc.vector.tensor_copy(out=dve_touch, in_=xin_s[:, 0:1])

        # stash regions (per j-block, the S columns future blocks mirror):
        # j=0 -> [0:384], j=1 -> [384:640], j=2 -> [640:768]
        stash = singles.tile([128, 768], F32)
        stash_off = [0, 384, 640]

        for qb in range(NB):
            kmin = qb * 128            # keys below kmin come from mirrors
            fd = N - kmin
            s0 = spool.tile([128, 512], F32, tag="s0")
            s1 = spool.tile([128, 512], F32, tag="s1")
            # mirror blocks (qb, j) for j < qb from stashed S^T source
            for j in range(qb):
                src_ap = stash[:, stash_off[j] + (qb - j - 1) * 128 :
                               stash_off[j] + (qb - j) * 128]
                wt_m = s0[:, j * 128 : (j + 1) * 128]
                nc.tensor.transpose(wt_m, src_ap, id_v)
            for s in range(PPB):
                t = qb * PPB + s
                lo = kmin + 2 * s       # first key this pair computes
                fds = N - lo
                t2 = t2pool.tile([P, N], F32R, tag="t2")
                nc.scalar.activation(
                    out=t2[:, 0:fds],
                    in_=xkT2_v[:, lo:N],
                    func=mybir.ActivationFunctionType.Tanh,
                    bias=xqB_v[:, t : t + 1],
                )
                ew = ebig_s[:, EBASE - 2 * s : EBASE - 2 * s + QB]
                nc.tensor.matmul(
                    out=s0[:, lo:512],
                    lhsT=ew,
                    rhs=t2[:, 0 : 512 - lo],
                    start=(s == 0),
                    stop=(s == PPB - 1),
                )
                nc.tensor.matmul(
                    out=s1,
                    lhsT=ew,
                    rhs=t2[:, 512 - lo : 1024 - lo],
                    start=(s == 0),
                    stop=(s == PPB - 1),
                )
            # fill the diagonal block's strict-lower part by a masked
            # transpose-accumulate of its upper part. The mask su_v zeroes
            # everything except source elements (r, c) with c > r, minus the
            # (r even, c == r+1) elements whose mirrors the odd rows already
            # computed directly.
            diagum = wpool.tile([128, 128], F32, tag="diagum")
            nc.vector.tensor_mul(out=diagum, in0=s0[:, kmin : kmin + 128], in1=su_v)
            nc.tensor.matmul(
                out=s0[:, kmin : kmin + 128],
                lhsT=diagum,
                rhs=id_v,
                is_transpose=True,
                start=False,
                stop=True,
                skip_group_check=True,
            )
            # stash the columns later blocks will mirror (ScalarE, PSUM src)
            if qb < NB - 1:
                nc.scalar.copy(
                    out=stash[:, stash_off[qb] : stash_off[qb] + 512 - kmin - 128],
                    in_=s0[:, kmin + 128 : 512],
                )

            # --- softmax over the 1024 keys ---
            # No max-subtraction needed: |S| <= D = 64 (sum of tanh), and
            # exp(64) ~ 6.2e27 fits fp32 with room to spare. (Also: ACT
            # accum_out is avoided -- its trailing accumulator write lands
            # after the instruction's semaphore fires, racing any prompt
            # consumer; and with a user bias AP it returns garbage
            # outright.) Z is computed with DVE reduce_sum instead.
            w0 = wpool.tile([128, 512], F32, tag="w0")
            w1 = wpool.tile([128, 512], F32, tag="w1")
            nc.scalar.activation(
                out=w0, in_=s0, func=mybir.ActivationFunctionType.Exp,
            )
            nc.scalar.activation(
                out=w1, in_=s1, func=mybir.ActivationFunctionType.Exp,
            )
            z0 = sm.tile([128, 1], F32, tag="z0")
            z1 = sm.tile([128, 1], F32, tag="z1")
            nc.vector.reduce_sum(out=z0, in_=w0, axis=mybir.AxisListType.X)
            nc.vector.reduce_sum(out=z1, in_=w1, axis=mybir.AxisListType.X)
            z = sm.tile([128, 1], F32, tag="z")
            nc.vector.tensor_add(out=z, in0=z0, in1=z1)
            rz = sm.tile([128, 1], F32, tag="rz")
            nc.vector.reciprocal(out=rz, in_=z)
            # pull rz's DVE tick into ACT's clock (so the av scale below
            # carries only its PE wait)
            rzt = sm.tile([128, 1], F32, tag="rzt")
            nc.scalar.copy(out=rzt, in_=rz)

            # --- W^T via PE transpose; PSUM->SBUF copies on ScalarE so the
            # wtps slot release merges with the transposes' ACT deps ---
            wt_s = wpool.tile([128, KC, 128], F32, tag="wt")
            for kc in range(KC):
                wt_p = wtps.tile([128, 128], F32, tag="wtp")
                wsrc = (w0 if kc < 4 else w1)[:, (kc % 4) * 128 : (kc % 4 + 1) * 128]
                nc.tensor.transpose(wt_p, wsrc, id_v)
                nc.scalar.copy(out=wt_s[:, kc, :], in_=wt_p)
            av = avps.tile([128, D], F32, tag="av")
            for kc in range(KC):
                nc.tensor.matmul(
                    out=av,
                    lhsT=wt_s[:, kc, :],
                    rhs=xkAV_v[:, kc, :],
                    start=(kc == 0),
                    stop=(kc == KC - 1),
                )
            o_s = obig[:, qb, :]
            # normalize on ACT at the [128, 64] output (cheaper than scaling
            # W): the rz read is cross-engine-synced via rzt above, and the
            # av read is PSUM (ScalarE's fast port)
            nc.scalar.mul(out=o_s, in_=av, mul=rz)
            # dummy PE read of o_s: pulls the DVE tick into PE's clock so the
            # next block's AV matmul (avps slot reuse) has only its ACT dep
            nc.tensor.transpose(junk, o_s[:, 0:2], o_s[:, 0:1])

        # single output DMA: out[qb*128 + p, d] = obig[p, qb, d]
        nc.sync.dma_start(
            out=out.rearrange("(nb p) d -> p nb d", p=128), in_=obig
        )

    _strip_self_waits(nc)
    return nc


# Engine's own-semaphore waits are redundant: ACT/DVE execute strictly
# in-order (one instruction at a time through the datapath, drained between),
# and PE instruction writes are pc-monotone (the only reorder is LDWEIGHTS
# pull-ahead, which reads SBUF that PE never writes). Tile emits them anyway
# for slot-reuse WAW, and walrus codegen rejects >1 sync wait per
# instruction, so strip them.
_SELF_SEM = {
    mybir.EngineType.Activation: "Activation_",
    mybir.EngineType.DVE: "DVE_",
    mybir.EngineType.PE: "PE_",
}


def _strip_self_waits(nc):
    # semaphores incremented by DMAs that write ExternalOutput DRAM: these
    # waits on the final drain are load-bearing (nothing else implies the
    # output transfer finished).
    out_queues = set()
    for inst in nc.inst_map.values():
        if "DMA" in type(inst).__name__.upper():
            outs = getattr(inst, "outs", None) or []
            for o in outs:
                if getattr(o, "memsetref", "") == "out_set":
                    si = inst.sync_info
                    for u in si.on_update if si else []:
                        out_queues.add(u.ant_name)

    for inst in nc.inst_map.values():
        si = inst.sync_info
        if si is None:
            continue
        tname = type(inst).__name__
        if tname == "InstDrain" and len(si.on_wait) > 1:
            # Kernel-tail join. Input-DMA / ACT / DVE completion is implied
            # transitively: every one of their results is consumed by a PE
            # instruction (junk transposes absorb the input DMAs; AV matmuls
            # consume ACT's last copies; the o_s junk transpose consumes
            # DVE's last write), and the per-engine follower drains empty
            # each engine's own pipeline. Keep only the PE count and the
            # output-DMA queue(s).
            # (the barrier gather phase already proves every engine drained
            # its own pipeline, which transitively covers input DMAs and all
            # compute sems -- only in-flight OUTPUT DMA completion is not
            # implied by anything else)
            kept = [w for w in si.on_wait if (w.ant_name or "") in out_queues]
            si.on_wait = kept[:1]
            continue
        eng = getattr(inst, "engine", None)
        prefix = _SELF_SEM.get(eng)
        if prefix is None:
            continue
        cross = [w for w in si.on_wait if not (w.ant_name or "").startswith(prefix)]
        if not cross:
            # self-waits only (real same-engine RAW ordering): keep them.
            if len(si.on_wait) > 1:
                raise AssertionError(f"{inst.name}: multiple self-waits")
            continue
        if len(si.on_wait) != len(cross):
            # self + cross: drop the self-waits. Only safe when the
            # self-dependency has instruction spacing (all such cases here
            # are slot-reuse WAW at distance >= 2 instructions).
            si.on_wait = cross
        if len(cross) > 1:
            raise AssertionError(
                f"{inst.name}: {len(cross)} cross-engine waits remain: "
                + ", ".join(f"{w.ant_name}>={w.wait_value}" for w in cross)
            )


_NC = None


_SU = None


def _su_mask():
    global _SU
    if _SU is None:
        r = np.arange(128)
        su = (r[None, :] > r[:, None]).astype(np.float32)   # c > r
        even = (r % 2 == 0)
        su[even, r[even] + 1] = 0.0     # odd rows computed (r+1, r) directly
        _SU = su
    return _SU


def _ebig_host():
    e = np.zeros((P, EW), dtype=np.float32)
    for p in range(P):
        e[p, EBASE + p // D] = 1.0
    return e


def kernel(inputs: np.ndarray) -> np.ndarray:
    global _NC
    x = np.ascontiguousarray(np.asarray(inputs, dtype=np.float32))
    assert x.shape == (B, N, D), x.shape
    if _NC is None:
        _NC = _build_bass()
    ebig_h = _ebig_host()
    ident_h = np.eye(128, dtype=np.float32)

    in_maps = []
    for c in range(NCORES):
        b, qh = divmod(c, 2)
        # permute keys so this core's own query half comes first: the
        # strip-diagonal block is then keys [0, 512) and the triangular
        # trimming + mirroring is the same SPMD program on every core.
        xk = np.concatenate(
            [x[b, qh * Q : (qh + 1) * Q], x[b, (1 - qh) * Q : (2 - qh) * Q]],
            axis=0,
        )                                          # (1024, 64) permuted
        xq = x[b, qh * Q : (qh + 1) * Q]           # (512, 64)
        xin = np.empty((P, XIN_W), dtype=np.float32)
        xin[:D, XKT2_OFF:XKT2_OFF + N] = xk.T
        xin[D:, XKT2_OFF:XKT2_OFF + N] = xk.T
        xin[:D, XQB_OFF:XQB_OFF + TP] = xq[0::2].T
        xin[D:, XQB_OFF:XQB_OFF + TP] = xq[1::2].T
        xin[:, ID_OFF:ID_OFF + 128] = ident_h
        xin[:, XKAV_OFF:XKAV_OFF + KC * D] = (
            xk.reshape(KC, 128, D).transpose(1, 0, 2).reshape(128, KC * D)
        )
        xin[:, SU_OFF:SU_OFF + 128] = _su_mask()
        in_maps.append(dict(xin=xin, ebig=ebig_h))

    res = run_bass_kernel_spmd(_NC, in_maps, core_ids=list(range(NCORES)))
    outs = [res.results[c]["out"] for c in range(NCORES)]
    return np.stack(
        [np.concatenate([outs[2 * b], outs[2 * b + 1]], axis=0) for b in range(B)],
        axis=0,
    )


if __name__ == "__main__":
    rng = np.random.default_rng(0)
    x = rng.standard_normal((B, N, D), dtype=np.float32)
    y = kernel(x)
    print(y.shape, y.dtype)
